# revision 1
# baseline (speedup 1.0000x reference)
"""Trainium2 Bass kernel v2 for the dense transformer block.

Sharding: 8-way SPMD, one (batch, half-sequence) shard of Tq=1024 query tokens
per core; K/V recomputed for the full batch (rows host-rolled so local queries
come first). No collectives.

v2 changes vs v1:
- Host-side folds: g1 into wq/wk rows (bq/bk = be1@wq carried as per-channel
  output biases), g2/be2 into w1/b1. LN kernels emit only u=(x-mu)*rstd.
- Phase 1 computes row stats lean (no Newton; ACT Rsqrt), normalizes on ACT
  (scale/bias per-partition), transposes fp16, and also transposes the raw
  rows of the query half once for the residual (kills v1 phase-5 reload).
- S matmuls (K=64) run as row-tiled head pairs: lhsT/rhs at partitions 0-63
  and 64-127 issue to disjoint PE row groups and execute concurrently (~2x).
- attn-proj also row-tiled: O^T stored as [128, Tq] fp16 head-pair tiles
  (odd head partition-shifted 0:64 -> 64:128 via tiny SBUF->SBUF DMA).
- ES/V are fp8 (SBUF relief; matmul rate unchanged), Q/K/O fp16.
- fc1 stays f32r, fc2 fp16 (DoubleRow fp8 measured no faster than f32r on
  this stack, so no fp8 GEMMs).
"""

import sys

import numpy as np

if "/opt/trn_rl_repo" not in sys.path:
    sys.path.insert(0, "/opt/trn_rl_repo")

CFG_FULL = dict(
    Tq=1024, Tkv=2048, C=1024, H=16, D=64, HID=4096, NCLS=1000, EPS=1e-5,
    B=4, N=2048,
)


def _chunks(total, size):
    out = []
    s = 0
    while s < total:
        c = min(size, total - s)
        out.append((s, c))
        s += c
    return out


def emit_block(tc, out_ap, ins, cfg):
    """Emit the full transformer-block program for one core's shard."""
    import concourse.mybir as mybir
    from concourse.masks import make_identity

    nc = tc.nc
    f32 = mybir.dt.float32
    f16 = mybir.dt.float16
    f8 = mybir.dt.float8e4
    f32r = mybir.dt.float32r
    AF = mybir.ActivationFunctionType
    OP = mybir.AluOpType
    AX = mybir.AxisListType

    Tq, Tkv, C, H, D = cfg["Tq"], cfg["Tkv"], cfg["C"], cfg["H"], cfg["D"]
    HID, NCLS, EPS = cfg["HID"], cfg["NCLS"], cfg["EPS"]
    P = 128
    CT = C // P           # 8 channel tiles
    KT = Tkv // P         # 16 key tiles
    HT = HID // P         # 32 hidden tiles
    HPAIR = H // 2        # 8 head pairs
    DA = D + 1
    SCALE = C ** -0.5
    gelu_func = AF.Tanh if cfg.get("sim_gelu_tanh") else AF.Gelu

    assert H * D == C and D == 64 and C % P == 0 and Tkv % P == 0

    def pool(name, bufs=1, space="SBUF"):
        return tc.tile_pool(name=name, bufs=bufs, space=space)

    # ---------------- constants & params ----------------
    const_cm = pool("const")
    const_pool = const_cm.__enter__()

    ident32 = const_pool.tile([P, P], f32)
    make_identity(nc, ident32)
    identr = const_pool.tile([P, P], f32r)
    nc.vector.tensor_copy(identr, ident32)
    ident16 = const_pool.tile([P, P], f16)
    nc.vector.tensor_copy(ident16, ident32)
    ident8 = const_pool.tile([P, P], f8)
    nc.vector.tensor_copy(ident8, ident32)
    ones16 = const_pool.tile([P, P], f16)
    nc.vector.memset(ones16, 1.0)
    ones_r = const_pool.tile([P, P], f32r)
    nc.vector.memset(ones_r.bitcast(f32), 1.0)
    eps_ap = const_pool.tile([P, 1], f32)
    nc.vector.memset(eps_ap, EPS)

    pp = {}
    with pool("ppps", bufs=2, space="PSUM") as psum_misc:
        def load_pp(vec_ap, n, key):
            nt = n // P
            ld = const_pool.tile([nt, P], f32, tag="pp_ld")
            nc.sync.dma_start(ld, vec_ap.rearrange("(a p) -> a p", p=P))
            ps = psum_misc.tile([P, nt], f32, tag="pp_ps")
            nc.tensor.matmul(ps, ld, ident32[:nt, :nt], is_transpose=True)
            dst = const_pool.tile([P, nt], f32, tag=f"pp_{key}")
            nc.vector.tensor_copy(dst, ps)
            pp[key] = dst

        for key in ["bq", "bk", "b_ap", "b2"]:
            load_pp(ins[key], C, key)
        load_pp(ins["b1"], HID, "b1")
    bout_sb = const_pool.tile([1, NCLS], f32r)
    nc.sync.dma_start(bout_sb, ins["b_out"][None, :].bitcast(f32r))

    # ---------------- long-lived arena ----------------
    arena_cm = pool("arena")
    arena = arena_cm.__enter__()

    def atile(slot, shape, dtype, name):
        return arena.tile(shape, dtype, tag=slot, name=name)

    # =========== Phase 1: rows -> LN1 stats -> xn^T (f16) + xq^T ===========
    # Single arena tiles [P, CT, T]; per-ct views keep downstream code shape.
    # Vp ones-columns preset here so the memsets run during the initial
    # DMA wait instead of stalling DVE at phase-3 start.
    Vp = [atile(f"v{i}", [P, 2, H, DA], f8, f"V{i}") for i in range(KT // 2)]
    for _vp in Vp:
        nc.vector.memset(_vp, 1.0)
    xq_all = atile("xq_all", [P, CT, Tq], f16, "xq_all")
    xn_all = atile("xn_all", [P, CT, Tkv], f16, "xn_all")
    xq_tiles = [xq_all[:, ct, :] for ct in range(CT)]
    xn_tiles = [xn_all[:, ct, :] for ct in range(CT)]

    # wq/wk staged + converted to fp16 during phase 1/2 (host pre-scaled by g1)
    wqk_cm = pool("wqk")
    wqk_pool = wqk_cm.__enter__()
    wst_cm = pool("wstage", bufs=2)
    wst_pool = wst_cm.__enter__()

    def stage_weight16(w_ap, key, engine, wpool=None):
        w16 = (wpool or wqk_pool).tile([P, CT, C], f16, tag=f"w16_{key}")
        wr = w_ap.rearrange("(ct p) m -> p ct m", p=P)
        for coff in range(0, C, 128):
            st = wst_pool.tile([P, CT, 128], f32, tag="wstage")
            nc.sync.dma_start(st, wr[:, :, coff:coff + 128])
            if engine == "dve":
                nc.vector.tensor_copy(w16[:, :, coff:coff + 128], st)
            else:
                nc.scalar.copy(w16[:, :, coff:coff + 128], st)
        return w16

    wq16 = stage_weight16(ins["wq"], "wq", "dve")
    wk16 = stage_weight16(ins["wk"], "wk", "act")
    QT = [atile(f"q{i}", [P, Tq], f16, f"QT{i}") for i in range(HPAIR)]
    KTt = [atile(f"x1_{i}", [P, Tkv], f16, f"KT{i}") for i in range(HPAIR)]

    with pool("ld1", bufs=2) as load_pool, \
         pool("lnst", bufs=4) as stat_pool, \
         pool("lnsq", bufs=2) as sq_pool, \
         pool("qkps", bufs=2, space="PSUM") as qk_psum, \
         pool("trps1", bufs=2, space="PSUM") as tpsum:

        def emit_qk_group(w16, dst, bias, goff):
            """One 1024-token group of Q- or K-gen for all 8 output tiles."""
            for m in range(CT):
                ps = qk_psum.tile([P, 2, 512], f32, tag="qk_ps")
                for i in range(2):
                    toff = goff + i * 512
                    for ct in range(CT):
                        nc.tensor.matmul(
                            ps[:, i, :], w16[:, ct, m * P:(m + 1) * P],
                            xn_tiles[ct][:, toff:toff + 512],
                            start=(ct == 0), stop=(ct == CT - 1))
                nc.vector.tensor_scalar_add(
                    dst[m][:, goff:goff + 1024],
                    ps.rearrange("p a b -> p (a b)"), bias[:, m:m + 1])

        def emit_p1_row(tg):
            r = load_pool.tile([P, C], f32, tag="rowload")
            nc.sync.dma_start(r, ins["xs"][tg * P:(tg + 1) * P, :])
            s1 = stat_pool.tile([P, 1], f32, tag="r_s1")
            nc.vector.reduce_sum(s1, r, axis=AX.X)
            sqt = sq_pool.tile([P, C], f32, tag="r_sq")
            s2 = stat_pool.tile([P, 1], f32, tag="r_s2")
            nc.vector.scalar_tensor_tensor(
                sqt, r, 1.0, r, OP.bypass, OP.mult, accum_out=s2)
            t = stat_pool.tile([P, 1], f32, tag="r_t")
            nc.vector.tensor_tensor(t, s1, s1, OP.mult)
            varp = stat_pool.tile([P, 1], f32, tag="r_varp")
            nc.vector.scalar_tensor_tensor(
                varp, t, -1.0 / C, s2, OP.mult, OP.add)
            std = stat_pool.tile([P, 1], f32, tag="r_std")
            nc.scalar.activation(std, varp, AF.Sqrt,
                                 bias=eps_ap, scale=1.0 / C)
            rstd = stat_pool.tile([P, 1], f32, tag="r_rstd")
            nc.vector.reciprocal(rstd, std)
            r16 = load_pool.tile([P, C], f16, tag="row16")
            if cfg.get("ln1_act_norm"):
                nb = stat_pool.tile([P, 1], f32, tag="r_nb")
                nc.vector.scalar_tensor_tensor(
                    nb, s1, -1.0 / C, rstd, OP.mult, OP.mult)
                nc.scalar.activation(r16, r, AF.Lrelu, bias=nb, scale=rstd,
                                     alpha=1.0)
            else:
                negmu = stat_pool.tile([P, 1], f32, tag="r_nb")
                nc.vector.tensor_scalar_mul(negmu, s1, -1.0 / C)
                nc.vector.tensor_scalar(r16, r, negmu, rstd, OP.add, OP.mult)
            # transpose normalized rows (fp16) -> xn^T (one copy per row tile)
            ps = tpsum.tile([P, CT, P], f16, tag="tr16")
            for j in range(CT):
                nc.tensor.matmul(
                    ps[:, j, :], r16[:, j * P:(j + 1) * P],
                    ident16, is_transpose=True,
                    start=(j == 0), stop=(j == CT - 1))
            nc.vector.tensor_copy(xn_all[:, :, tg * P:(tg + 1) * P], ps)
            # transpose raw rows (query half) -> xq^T fp16
            if tg < Tq // P:
                rq16 = load_pool.tile([P, C], f16, tag="rowq16")
                nc.vector.tensor_copy(rq16, r)
                ps = tpsum.tile([P, CT, P], f16, tag="trq16")
                for j in range(CT):
                    nc.tensor.matmul(
                        ps[:, j, :], rq16[:, j * P:(j + 1) * P],
                        ident16, is_transpose=True,
                        start=(j == 0), stop=(j == CT - 1))
                nc.vector.tensor_copy(xq_all[:, :, tg * P:(tg + 1) * P], ps)

        # interleave: rows 0-7, then Q + K(first half) overlap rows 8-15,
        # then K(second half)
        for tg in range(8):
            emit_p1_row(tg)
        emit_qk_group(wq16, QT, pp["bq"], 0)
        for tg in range(8, 12):
            emit_p1_row(tg)
        emit_qk_group(wk16, KTt, pp["bk"], 0)
        for tg in range(12, KT):
            emit_p1_row(tg)
        if cfg.get("stop_after") != 1:
            emit_qk_group(wk16, KTt, pp["bk"], 1024)

    wst_cm.__exit__(None, None, None)
    wqk_cm.__exit__(None, None, None)

    if cfg.get("stop_after") == 1:
        arena_cm.__exit__(None, None, None)
        const_cm.__exit__(None, None, None)
        return

    if cfg.get("stop_after") == 2:
        arena_cm.__exit__(None, None, None)
        const_cm.__exit__(None, None, None)
        return

    # ========== Phase 3: V (fp8, [k, head, d+ones]) ==========
    with pool("wv8p") as wv_pool, \
         pool("wv8st", bufs=1) as wv_stage, \
         pool("vld", bufs=6) as vload, \
         pool("vt", bufs=2) as vt_pool, \
         pool("vtps", bufs=3, space="PSUM") as vt_psum, \
         pool("vps", bufs=2, space="PSUM") as v_psum:
        wv16 = wv_pool.tile([P, CT, C], f16, tag="wv16")
        wvr = ins["wv"].rearrange("(ct p) m -> p ct m", p=P)
        for coff in range(0, C, 128):
            wvst = wv_stage.tile([P, CT, 128], f32, tag="wv_st")
            nc.sync.dma_start(wvst, wvr[:, :, coff:coff + 128])
            nc.scalar.copy(wv16[:, :, coff:coff + 128], wvst)
        def emit_vt(kt):
            rows = vload.tile([P, C], f32, tag="vrow")
            nc.sync.dma_start(rows, ins["vals"][kt * P:(kt + 1) * P, :])
            r16 = vload.tile([P, C], f16, tag="vrow16")
            nc.scalar.copy(r16, rows)
            vT16 = vt_pool.tile([P, CT, P], f16, tag="vT16")
            ps = vt_psum.tile([P, CT, P], f16, tag="vt_ps")
            for j in range(CT):
                nc.tensor.matmul(
                    ps[:, j, :], r16[:, j * P:(j + 1) * P],
                    ident16, is_transpose=True,
                    start=(j == 0), stop=(j == CT - 1))
            nc.vector.tensor_copy(vT16, ps)
            return vT16

        def emit_vmm(kt, vT16):
            vp = v_psum.tile([P, 2, 512], f32, tag="v_ps")
            for i, (noff, nsz) in enumerate(_chunks(C, 512)):
                for ct in range(CT):
                    nc.tensor.matmul(
                        vp[:, i, :], vT16[:, ct, :],
                        wv16[:, ct, noff:noff + nsz],
                        start=(ct == 0), stop=(ct == CT - 1))
            nc.vector.tensor_copy(
                Vp[kt // 2][:, kt % 2, :, 0:D],
                vp.rearrange("p a (h d) -> p (a h) d", d=D))

        prev = emit_vt(0)
        for kt in range(1, KT):
            cur = emit_vt(kt)
            emit_vmm(kt - 1, prev)
            prev = cur
        emit_vmm(KT - 1, prev)

    if cfg.get("stop_after") == 3:
        arena_cm.__exit__(None, None, None)
        const_cm.__exit__(None, None, None)
        return

    # ====== Phase 4: attention (S^T row-tiled pairs; fp8 ES/V) ======
    OT = [atile(f"o{i}", [P, Tq], f16, f"OT{i}") for i in range(HPAIR)]
    # w_ap fp16 conversion early (DVE) so phase 5 can start immediately
    wap_cm = pool("wapp")
    wap_pool = wap_cm.__enter__()
    wap16 = wap_pool.tile([P, CT, C], f16, tag="wap16")
    wapr = ins["w_ap"].rearrange("(ct p) m -> p ct m", p=P)
    for coff in range(0, C, 128):
        wapst = wap_pool.tile([P, CT, 128], f32, tag="wap_st")
        nc.sync.dma_start(wapst, wapr[:, :, coff:coff + 128])
        nc.vector.tensor_copy(wap16[:, :, coff:coff + 128], wapst)

    # Schraudolph fast-exp constants (exp via int32 bit pattern); DVE handles
    # a share of the exp tiles so ACT (the phase bottleneck) is relieved.
    EXPA = float((2 ** 23) * SCALE / np.log(2.0))
    EXPB = float(127 * (2 ** 23) - 0.0579 * (2 ** 23))
    exp_dve_mod = cfg.get("exp_dve_mod", 3)
    i32 = mybir.dt.int32

    with pool("es", bufs=2) as es_pool, \
         pool("attsmall", bufs=2) as small_pool, \
         pool("expdve", bufs=1) as expdve_pool, \
         pool("sps", bufs=2, space="PSUM") as s_psum, \
         pool("ops", bufs=2, space="PSUM") as o_psum, \
         pool("bcps", bufs=2, space="PSUM") as bc_psum:

        def emit_s_exp(hp, qoff, qsz):
            es = es_pool.tile([P, 2, KT, qsz], f8, tag="es")
            for kt in range(KT):
                sp = s_psum.tile([P, 2, qsz], f32, tag="s_ps")
                nc.tensor.matmul(
                    sp[:, 0, :], KTt[hp][0:64, kt * P:(kt + 1) * P],
                    QT[hp][0:64, qoff:qoff + qsz],
                    start=True, stop=True)
                nc.tensor.matmul(
                    sp[:, 1, :], KTt[hp][64:128, kt * P:(kt + 1) * P],
                    QT[hp][64:128, qoff:qoff + qsz],
                    start=True, stop=True)
                if exp_dve_mod and kt % exp_dve_mod == exp_dve_mod - 1:
                    ti = expdve_pool.tile([P, 2, qsz], i32, tag="exp_ti")
                    nc.vector.tensor_scalar(
                        ti, sp, EXPA, EXPB, OP.mult, OP.add)
                    nc.vector.tensor_copy(
                        es[:, :, kt, :], ti.bitcast(f32))
                else:
                    nc.scalar.activation(es[:, :, kt, :], sp, AF.Exp,
                                         bias=0.0, scale=SCALE)
            return es

        def emit_av(hp, qoff, qsz, es):
            for j in (0, 1):
                h = 2 * hp + j
                op = o_psum.tile([DA, qsz], f32, tag="o_ps")
                for kt in range(KT):
                    nc.tensor.matmul(
                        op, Vp[kt // 2][:, kt % 2, h, :], es[:, j, kt, :],
                        start=(kt == 0), stop=(kt == KT - 1))
                rec = small_pool.tile([DA, qsz], f32, tag="rec")
                nc.vector.reciprocal(rec[D:DA, :], op[D:DA, :])
                recr = small_pool.tile([DA, qsz], f16, tag="recr")
                nc.vector.tensor_copy(recr[D:DA, :], rec[D:DA, :])
                bc = bc_psum.tile([D, qsz], f32, tag="bc")
                nc.tensor.matmul(bc, ones16[D:D + 1, 0:D],
                                 recr[D:DA, :], start=True, stop=True)
                bc_sb = small_pool.tile([D, qsz], f32, tag="bc_sb")
                nc.vector.tensor_copy(bc_sb, bc)
                if j == 0:
                    nc.vector.tensor_tensor(
                        OT[hp][0:64, qoff:qoff + qsz], op[0:D, :],
                        bc_sb, OP.mult)
                else:
                    otmp = small_pool.tile([D, qsz], f16, tag="otmp")
                    nc.vector.tensor_tensor(
                        otmp, op[0:D, :], bc_sb, OP.mult)
                    nc.sync.dma_start(
                        OT[hp][64:128, qoff:qoff + qsz], otmp)

        # pipeline: exp of unit u+1 (ACT) overlaps AV of unit u (PE)
        units = [(hp, qoff, qsz) for hp in range(HPAIR)
                 for (qoff, qsz) in _chunks(Tq, 512)]
        prev_unit, prev_es = units[0], emit_s_exp(*units[0])
        for unit in units[1:]:
            cur_es = emit_s_exp(*unit)
            emit_av(*prev_unit, prev_es)
            prev_unit, prev_es = unit, cur_es
        emit_av(*prev_unit, prev_es)

    if cfg.get("stop_after") == 4:
        wap_cm.__exit__(None, None, None)
        wqk_cm.__exit__(None, None, None)
        arena_cm.__exit__(None, None, None)
        const_cm.__exit__(None, None, None)
        return

    # ========= Phase 5: attn proj (row-tiled pairs) + residual 1 =========
    x1 = [atile(f"x1_{ct}", [P, Tq], f32r, f"x1_{ct}") for ct in range(CT)]
    with pool("apps", bufs=2, space="PSUM") as ap_psum, \
         pool("apsb", bufs=2) as ap_sb:
        for ct in range(CT):
            psA = ap_psum.tile([P, 2, 512], f32, tag="ap_psA")
            psB = ap_psum.tile([P, 2, 512], f32, tag="ap_psB")
            for j, (toff, tsz) in enumerate(_chunks(Tq, 512)):
                for i in range(HPAIR):
                    nc.tensor.matmul(
                        psA[:, j, :], wap16[0:64, i, ct * P:(ct + 1) * P],
                        OT[i][0:64, toff:toff + tsz],
                        start=(i == 0), stop=(i == HPAIR - 1))
                    nc.tensor.matmul(
                        psB[:, j, :], wap16[64:128, i, ct * P:(ct + 1) * P],
                        OT[i][64:128, toff:toff + tsz],
                        start=(i == 0), stop=(i == HPAIR - 1))
            tA = ap_sb.tile([P, Tq], f32, tag="ap_tA")
            nc.vector.scalar_tensor_tensor(
                tA, psA.rearrange("p a b -> p (a b)"),
                pp["b_ap"][:, ct:ct + 1], xq_tiles[ct], OP.add, OP.add)
            nc.vector.tensor_tensor(
                x1[ct], tA, psB.rearrange("p a b -> p (a b)"), OP.add)
    wap_cm.__exit__(None, None, None)

    if cfg.get("stop_after") == 5:
        arena_cm.__exit__(None, None, None)
        const_cm.__exit__(None, None, None)
        return

    # ============ Phase 6+7: LN2 (u only), fc1 + gelu -> h^T fp16 ============
    with pool("ln2ps", bufs=2, space="PSUM") as ln_spool, \
         pool("ln2stat", bufs=2) as ln_stat, \
         pool("w1", bufs=2) as w1_pool, \
         pool("f1ps", bufs=2, space="PSUM") as f1_psum:
        xn2_all = atile("xn_all", [P, CT, Tq], f32r, "xn2_all")
        xn2 = [xn2_all[:, ct, :] for ct in range(CT)]
        for (toff, tsz) in _chunks(Tq, 512):
            s1 = ln_spool.tile([P, tsz], f32, tag="ln_s1")
            s2 = ln_spool.tile([P, tsz], f32, tag="ln_s2")
            for ct in range(CT):
                xc = x1[ct][:, toff:toff + tsz]
                nc.tensor.matmul(s1, ones_r, xc,
                                 start=(ct == 0), stop=(ct == CT - 1))
                sq = ln_stat.tile([P, tsz], f32r, tag="ln_sq")
                nc.vector.tensor_tensor(
                    sq, xc.bitcast(f32), xc.bitcast(f32), OP.mult)
                nc.tensor.matmul(s2, ones_r, sq,
                                 start=(ct == 0), stop=(ct == CT - 1))
            mu = ln_stat.tile([P, tsz], f32, tag="ln_mu")
            nc.vector.tensor_scalar_mul(mu, s1, 1.0 / C)
            m2 = ln_stat.tile([P, tsz], f32, tag="ln_m2")
            nc.vector.tensor_scalar_mul(m2, s2, 1.0 / C)
            musq = ln_stat.tile([P, tsz], f32, tag="ln_musq")
            nc.vector.tensor_tensor(musq, mu, mu, OP.mult)
            var = ln_stat.tile([P, tsz], f32, tag="ln_var")
            nc.vector.tensor_tensor(var, m2, musq, OP.subtract)
            std = ln_stat.tile([P, tsz], f32, tag="ln_std")
            nc.scalar.activation(std, var, AF.Sqrt,
                                 bias=eps_ap, scale=1.0)
            A = ln_stat.tile([P, tsz], f32, tag="ln_A")
            nc.vector.reciprocal(A, std)
            Bt = ln_stat.tile([P, tsz], f32, tag="ln_B")
            nc.vector.scalar_tensor_tensor(
                Bt, mu, -1.0, A, OP.mult, OP.mult)
            for ct in range(CT):
                u = ln_stat.tile([P, tsz], f32, tag="ln_u")
                nc.vector.tensor_tensor(
                    u, x1[ct].bitcast(f32)[:, toff:toff + tsz], A, OP.mult)
                nc.vector.tensor_tensor(
                    xn2[ct][:, toff:toff + tsz], u, Bt, OP.add)

        hT_x = atile("xq_all", [P, 8, Tq], f16, "hT_x")
        hT = ([atile(f"q{i}", [P, Tq], f16, f"hTq{i}") for i in range(8)]
              + [atile(f"v{i}", [P, Tq], f16, f"hTv{i}") for i in range(8)]
              + [atile(f"o{i}", [P, Tq], f16, f"hTo{i}") for i in range(8)]
              + [hT_x[:, i, :] for i in range(8)])
        w1r = ins["w1"].rearrange("(ct p) m -> p ct m", p=P)
        W1CH = 512
        for (moff, msz) in _chunks(HID, W1CH):
            w1_cb = w1_pool.tile([P, CT, W1CH], f32r, tag="w1cb")
            nc.gpsimd.dma_start(
                w1_cb[:, :, :msz],
                w1r[:, :, moff:moff + msz].bitcast(f32r))
            for mi in range(msz // P):
                m = (moff + mi * P) // P
                ps = f1_psum.tile([P, 2, 512], f32, tag="f1_ps")
                for i, (toff, tsz) in enumerate(_chunks(Tq, 512)):
                    for ct in range(CT):
                        nc.tensor.matmul(
                            ps[:, i, :], w1_cb[:, ct, mi * P:(mi + 1) * P],
                            xn2[ct][:, toff:toff + tsz],
                            start=(ct == 0), stop=(ct == CT - 1))
                nc.scalar.activation(
                    hT[m], ps.rearrange("p a b -> p (a b)"), gelu_func,
                    bias=pp["b1"][:, m:m + 1], scale=1.0)

    if cfg.get("stop_after") == 7:
        arena_cm.__exit__(None, None, None)
        const_cm.__exit__(None, None, None)
        return

    # ================= Phase 8: fc2 + residual 2 (in-place x1) =================
    with pool("w2f", bufs=2) as w2f_pool, \
         pool("w2h", bufs=2) as w2h_pool, \
         pool("f2ps", bufs=2, space="PSUM") as f2_psum:
        w2r = ins["w2"].rearrange("(ht p) c -> p ht c", p=P)
        for ct in range(CT):
            w2_f32 = w2f_pool.tile([P, HT, P], f32, tag="w2f32")
            nc.gpsimd.dma_start(w2_f32, w2r[:, :, ct * P:(ct + 1) * P])
            w2_f16 = w2h_pool.tile([P, HT, P], f16, tag="w2f16")
            nc.vector.tensor_copy(w2_f16, w2_f32)
            ps = f2_psum.tile([P, 2, 512], f32, tag="f2_ps")
            for i, (toff, tsz) in enumerate(_chunks(Tq, 512)):
                for ht in range(HT):
                    nc.tensor.matmul(
                        ps[:, i, :], w2_f16[:, ht, :],
                        hT[ht][:, toff:toff + tsz],
                        start=(ht == 0), stop=(ht == HT - 1))
            nc.vector.scalar_tensor_tensor(
                x1[ct], ps.rearrange("p a b -> p (a b)"),
                pp["b2"][:, ct:ct + 1],
                x1[ct].bitcast(f32), OP.add, OP.add)

    if cfg.get("stop_after") == 8:
        arena_cm.__exit__(None, None, None)
        const_cm.__exit__(None, None, None)
        return

    # ================= Phase 9: out proj + softmax =================
    with pool("wout") as wpool, \
         pool("smax", bufs=3) as sm_pool, \
         pool("smsmall", bufs=6) as sms_pool, \
         pool("outps", bufs=4, space="PSUM") as out_psum:
        wout_sb = wpool.tile([P, CT, NCLS], f32r, tag="wout")
        nc.sync.dma_start(
            wout_sb,
            ins["w_out"].rearrange("(ct p) m -> p ct m", p=P).bitcast(f32r))
        for tt in range(Tq // P):
            # bank-aligned [P, 2, 512] psum; logits occupy [:, :, 0:500]
            ps = out_psum.tile([P, 2, 512], f32, tag="out_ps")
            for i, (noff, nsz) in enumerate(_chunks(NCLS, 500)):
                for ct in range(CT):
                    nc.tensor.matmul(
                        ps[:, i, 0:nsz], x1[ct][:, tt * P:(tt + 1) * P],
                        wout_sb[:, ct, noff:noff + nsz],
                        start=(ct == 0), stop=False)
                nc.tensor.matmul(
                    ps[:, i, 0:nsz], ones_r[0:1, :],
                    bout_sb[0:1, noff:noff + nsz],
                    start=False, stop=True)
            mx = sms_pool.tile([P, 2], f32, tag="sm_mx")
            nc.vector.reduce_max(mx, ps[:, :, 0:500], axis=AX.X)
            m = sms_pool.tile([P, 1], f32, tag="sm_m")
            nc.vector.reduce_max(m, mx, axis=AX.X)
            negm = sms_pool.tile([P, 1], f32, tag="sm_negm")
            nc.vector.tensor_scalar_mul(negm, m, -1.0)
            esb = sm_pool.tile([P, NCLS], f32, tag="sm_e")
            s = sms_pool.tile([P, 1], f32, tag="sm_s")
            nc.scalar.activation(
                esb.rearrange("p (a b) -> p a b", a=2), ps[:, :, 0:500],
                AF.Exp, bias=negm, scale=1.0, accum_out=s)
            rec = sms_pool.tile([P, 1], f32, tag="sm_rec")
            nc.vector.reciprocal(rec, s)
            nc.vector.tensor_scalar_mul(esb, esb, rec)
            nc.sync.dma_start(out_ap[tt * P:(tt + 1) * P, :], esb)

    arena_cm.__exit__(None, None, None)
    const_cm.__exit__(None, None, None)


# ======================= host entry =======================

_IN_NAMES = ["xs", "vals", "wq", "wk", "wv", "w_ap", "b_ap", "bq", "bk",
             "w1", "b1", "w2", "b2", "w_out", "b_out"]


def _build_nc(cfg):
    import concourse.bacc as bacc
    import concourse.mybir as mybir
    import concourse.tile as tile

    Tq, Tkv, C = cfg["Tq"], cfg["Tkv"], cfg["C"]
    HID, NCLS = cfg["HID"], cfg["NCLS"]
    shapes = dict(
        xs=[Tkv, C], vals=[Tkv, C], wq=[C, C], wk=[C, C], wv=[C, C],
        w_ap=[C, C], b_ap=[C], bq=[C], bk=[C],
        w1=[C, HID], b1=[HID], w2=[HID, C], b2=[C],
        w_out=[C, NCLS], b_out=[NCLS],
    )
    nc = bacc.Bacc("TRN2", target_bir_lowering=False, debug=False)
    ins = {k: nc.dram_tensor(k, shapes[k], mybir.dt.float32,
                             kind="ExternalInput").ap()
           for k in _IN_NAMES}
    out_ap = nc.dram_tensor("out", [Tq, NCLS], mybir.dt.float32,
                            kind="ExternalOutput").ap()
    with tile.TileContext(nc) as tc:
        emit_block(tc, out_ap, ins, cfg)
    nc.finalize()
    return nc


def make_in_maps(inputs, cfg):
    """Host-side prep: weight folds (g1->wq/wk + bq/bk, g2/be2->w1/b1) and
    per-core row rolls. Returns list of 8 input dicts."""
    B, N, Tq = cfg["B"], cfg["N"], cfg["Tq"]
    halves = N // Tq
    f = lambda k: np.asarray(inputs[k], np.float32)
    g1, be1 = f("g1"), f("be1")
    g2, be2 = f("g2"), f("be2")
    wq, wk, w1 = f("wq"), f("wk"), f("w1")
    shared = {
        "wq": np.ascontiguousarray(g1[:, None] * wq),
        "wk": np.ascontiguousarray(g1[:, None] * wk),
        "bq": np.ascontiguousarray(be1 @ wq),
        "bk": np.ascontiguousarray(be1 @ wk),
        "w1": np.ascontiguousarray(g2[:, None] * w1),
        "b1": np.ascontiguousarray(f("b1") + be2 @ w1),
        "wv": f("wv"), "w_ap": f("w_ap"), "b_ap": f("b_ap"),
        "w2": f("w2"), "b2": f("b2"),
        "w_out": f("w_out"), "b_out": f("b_out"),
    }
    shared = {k: np.ascontiguousarray(v) for k, v in shared.items()}
    x = np.ascontiguousarray(f("x"))
    value = np.ascontiguousarray(f("value"))
    in_maps = []
    for core in range(8):
        b, hf = core // halves, core % halves
        m = dict(shared)
        m["xs"] = np.ascontiguousarray(np.roll(x[b], -hf * Tq, axis=0))
        m["vals"] = np.ascontiguousarray(np.roll(value[b], -hf * Tq, axis=0))
        in_maps.append(m)
    return in_maps


_NC_CACHE = {}


def kernel(**inputs) -> np.ndarray:
    from concourse.bass_utils import run_bass_kernel_spmd

    cfg = CFG_FULL
    B, N = cfg["B"], cfg["N"]
    Tq, NCLS = cfg["Tq"], cfg["NCLS"]
    halves = N // Tq

    if "full" not in _NC_CACHE:
        _NC_CACHE["full"] = _build_nc(cfg)
    nc = _NC_CACHE["full"]

    in_maps = make_in_maps(inputs, cfg)
    res = run_bass_kernel_spmd(nc, in_maps, core_ids=list(range(8)))
    out = np.empty((B, N, NCLS), dtype=np.float32)
    for core in range(8):
        b, hf = core // halves, core % halves
        out[b, hf * Tq:(hf + 1) * Tq, :] = res.results[core]["out"]
    return out

